# revision 12
# baseline (speedup 1.0000x reference)
"""Trainium2 Bass kernel for nn_EquivariantVelocityHead.

Full-input contract: kernel(**inputs) takes the unsharded inputs (as in
setup_inputs()) and returns the full [B*N, 3] output. Internally shards
data-parallel over the graph dimension B across 8 NeuronCores (all pairwise
interactions are intra-graph), with the tiny phi-MLP weights replicated.

Math (per graph, N=256 nodes, H=128):
  A = h @ W1[:H];  Bm = h @ W1[H:2H];  wd = W1[2H];  (phi layer 1 split)
  pre[p,q,:] = A[q] + Bm[p] + dist[p,q]*wd + b1
  coeff[p,q] = silu(pre) @ W2 + b2
  v[p] = sum_q coeff[p,q] * (pos[q] - pos[p])
       = coeff @ pos - rowsum(coeff) * pos[p]

Device layout: pre is materialized transposed [h=128 partitions, (p,q) free].
The B+dist term lands in PSUM via one K=3 bf16 matmul per node pair:
stationary rows [wd; B[u]; B[u+128]] (built on device, staged to
partitions 0-2), moving rows [dist; ind_even; ind_odd] where dist rows are
DMA-staged onto partition 0 and the indicator rows select which half of the
block each B row lands in. The A-term (A^T = Wa^T hT, p-independent) is
computed ONCE on the PE, then broadcast-added to the PSUM pre on the
Pool/Vector engines (split between them) into an SBUF silu-input — this
removes a third of the PE matmul columns, which matters because the PE is
power-throttled when overdriven. Silu+b1 runs on ScalarE over 2048-col
tiles writing bf16. The W2 contraction uses a sliding-window stationary
(zeros | W2-column | zeros) so node u's pair accumulates into PSUM
partition u, yielding coeff[p-part, q-free] tiles directly; the final
contraction runs on VectorE accumulating reduces.
"""
import sys

sys.path.insert(0, "/opt/trn_rl_repo")

import numpy as np

B, N, H = 8, 256, 128
NCORES = 8

MMDT = "bf16"  # matmul operand dtype: "bf16" or "f32r"

_cache = {}


def _build(reps=1, variant="full", mmdt=MMDT):
    import concourse.bacc as bacc
    import concourse.mybir as mybir
    import concourse.tile as tile

    F32 = mybir.dt.float32
    MDT = mybir.dt.bfloat16 if mmdt == "bf16" else mybir.dt.float32r
    Alu = mybir.AluOpType
    Act = mybir.ActivationFunctionType

    nc = bacc.Bacc()

    hT_d = nc.declare_dram_parameter("hT", [H, N], MDT, isOutput=False)
    pos_d = nc.declare_dram_parameter("pos", [N, 3], F32, isOutput=False)
    rep_d = nc.declare_dram_parameter("rep3", [3, 128, N], F32, isOutput=False)
    wa_d = nc.declare_dram_parameter("wa", [H, H], MDT, isOutput=False)
    wb_d = nc.declare_dram_parameter("wb", [H, H], MDT, isOutput=False)
    wdrep_d = nc.declare_dram_parameter("wdrep", [1, 64 * H], MDT,
                                        isOutput=False)
    ind_d = nc.declare_dram_parameter("ind", [4, 2048], MDT, isOutput=False)
    zw_d = nc.declare_dram_parameter("zw", [H, 2 * H], MDT, isOutput=False)
    b1c_d = nc.declare_dram_parameter("b1c", [H, 1], F32, isOutput=False)
    b2c_d = nc.declare_dram_parameter("b2c", [128, 1], F32, isOutput=False)
    v_d = nc.declare_dram_parameter("v", [N, 3], F32, isOutput=True)

    with tile.TileContext(nc) as tc:
        with (
            tc.tile_pool(name="const", bufs=1) as cpool,
            tc.tile_pool(name="work", bufs=2) as wpool,
            tc.tile_pool(name="stage", bufs=8) as spool,
            tc.tile_pool(name="silin", bufs=3) as npool,
            tc.tile_pool(name="silu", bufs=4) as lpool,
            tc.tile_pool(name="fin", bufs=2) as fpool,
            tc.tile_pool(name="pre", bufs=3, space="PSUM") as pre_pool,
            tc.tile_pool(name="cps", bufs=1, space="PSUM") as cps_pool,
            tc.tile_pool(name="bps", bufs=1, space="PSUM") as bps_pool,
        ):
            # ---- constants / inputs ----
            hT = cpool.tile([H, N], MDT, tag="hT")
            nc.sync.dma_start(hT[:], hT_d[:])
            wa = cpool.tile([H, H], MDT, tag="wa")
            nc.sync.dma_start(wa[:], wa_d[:])
            wb = cpool.tile([H, H], MDT, tag="wb")
            nc.sync.dma_start(wb[:], wb_d[:])
            zw = cpool.tile([H, 2 * H], MDT, tag="zw")
            nc.sync.dma_start(zw[:], zw_d[:])
            b1c = cpool.tile([H, 1], F32, tag="b1c")
            nc.sync.dma_start(b1c[:], b1c_d[:])
            b2c = cpool.tile([128, 1], F32, tag="b2c")
            nc.sync.dma_start(b2c[:], b2c_d[:])
            rep = []
            for a in range(3):
                r = cpool.tile([128, N], F32, tag=f"rep{a}")
                nc.sync.dma_start(r[:], rep_d[a])
                rep.append(r)
            pcol = []
            for t in range(2):
                p = cpool.tile([128, 3], F32, tag=f"pcol{t}")
                nc.sync.dma_start(p[:], pos_d[128 * t:128 * (t + 1), :])
                pcol.append(p)
            bwd = cpool.tile([5, 64 * H], MDT, tag="bwd")
            nc.sync.dma_start(bwd[0:1, :], wdrep_d[:])
            at_sb = cpool.tile([128, N], F32, tag="at")

            for rp in range(reps):
                # ---- Bm = h @ Wb, scattered into bwd partitions 1-2 ----
                bsb = []
                for t in range(2):
                    bp = bps_pool.tile([128, H], F32, tag="bps")
                    nc.tensor.matmul(bp[:], hT[:, 128 * t:128 * (t + 1)],
                                     wb[:], start=True, stop=True,
                                     skip_group_check=True)
                    bs = wpool.tile([128, H], MDT, tag="bsb",
                                    name=f"bsb{t}_{rp}")
                    nc.vector.tensor_copy(bs[:], bp[:])
                    bsb.append(bs)
                for t in range(2):
                    par = bsb[t][:].rearrange("(a two) c -> two a c", two=2)
                    for j in range(2):
                        dst = bwd[1 + t + 2 * j:2 + t + 2 * j, :]
                        nc.sync.dma_start(
                            dst.rearrange("o (a c) -> o a c", c=H), par[j])

                # ---- A^T = Wa^T @ hT once per graph ([h-out, q]) ----
                for t in range(2):
                    ap_ = bps_pool.tile([128, H], F32, tag="bps")
                    nc.tensor.matmul(ap_[:], wa[:],
                                     hT[:, 128 * t:128 * (t + 1)],
                                     start=True, stop=True,
                                     skip_group_check=True)
                    nc.vector.tensor_copy(at_sb[:, 128 * t:128 * (t + 1)],
                                          ap_[:])

                # ---- dist tiles [p-part, q-free], exact diff formulation ----
                dist = []
                for t in range(2):
                    dx = wpool.tile([128, N], F32, tag="dx", name=f"dx{t}_{rp}")
                    dy = wpool.tile([128, N], F32, tag="dy", name=f"dy{t}_{rp}")
                    dz = wpool.tile([128, N], F32, tag="dz", name=f"dz{t}_{rp}")
                    nc.gpsimd.tensor_scalar(dx[:], rep[0][:], pcol[t][:, 0:1],
                                            None, Alu.subtract)
                    nc.gpsimd.tensor_scalar(dy[:], rep[1][:], pcol[t][:, 1:2],
                                            None, Alu.subtract)
                    nc.gpsimd.tensor_scalar(dz[:], rep[2][:], pcol[t][:, 2:3],
                                            None, Alu.subtract)
                    sx = wpool.tile([128, N], F32, tag="sx", name=f"sx{t}_{rp}")
                    sy = wpool.tile([128, N], F32, tag="sy", name=f"sy{t}_{rp}")
                    nc.gpsimd.tensor_tensor(sx[:], dx[:], dx[:], Alu.mult)
                    nc.gpsimd.tensor_tensor(sy[:], dy[:], dy[:], Alu.mult)
                    nc.gpsimd.tensor_tensor(sx[:], sx[:], sy[:], Alu.add)
                    nc.gpsimd.tensor_tensor(sy[:], dz[:], dz[:], Alu.mult)
                    nc.gpsimd.tensor_tensor(sx[:], sx[:], sy[:], Alu.add)
                    dt_ = wpool.tile([128, N], MDT, tag="dist",
                                     name=f"dist{t}_{rp}")
                    nc.scalar.activation(dt_[:], sx[:], Act.Sqrt)
                    dist.append(dt_)

                # ---- stage: [dist rows; indicators] on partitions 0-2 ----
                stages = []
                for c in range(32):
                    st = spool.tile([5, 8 * N], MDT, tag="stage",
                                    name=f"stage{c}_{rp}")
                    row = st[0:1, :].rearrange("o (r two q) -> o r two q",
                                               two=2, q=N)
                    nc.sync.dma_start(row[:, :, 0, :],
                                      dist[0][4 * c:4 * c + 4, :])
                    nc.sync.dma_start(row[:, :, 1, :],
                                      dist[1][4 * c:4 * c + 4, :])
                    nc.sync.dma_start(st[1:5, :], ind_d[:, :])
                    stages.append(st)

                # ---- coeff accumulator: cols 0:256 -> nodes 0..127,
                # cols 256:512 -> nodes 128..255 ----
                cps = cps_pool.tile([128, 2 * N], F32, tag="c",
                                    name=f"cps_{rp}")

                # ---- main loop: 64 blocks of 2 node-pairs; silu per 2
                # blocks (2048-col ACT tiles) ----
                def emit_w2(pair):
                    sil = sils[pair]
                    for j in range(4):
                        u = 4 * pair + j
                        nc.tensor.matmul(cps[:], zw[:, 128 - u:256 - u],
                                         sil[:, 512 * j:512 * j + 512],
                                         start=(u == 0), stop=(u == 127),
                                         skip_group_check=True)

                sils = {}
                silin = None
                for ob in range(64):
                    pre = pre_pool.tile([128, 1024], F32, tag="pre",
                                        name=f"pre{ob}_{rp}")
                    st = stages[ob // 2]
                    for hh in range(2):
                        off = (ob % 2) * 1024 + 512 * hh
                        nc.tensor.matmul(pre[:, 512 * hh:512 * hh + 512],
                                         bwd[0:5, ob * H:(ob + 1) * H],
                                         st[0:5, off:off + 512], start=True,
                                         stop=True, skip_group_check=True)
                    pair, half = divmod(ob, 2)
                    if half == 0:
                        silin = npool.tile([128, 2048], F32, tag="silin",
                                           name=f"silin{pair}_{rp}")
                    dst = silin[:, 1024 * half:1024 * half + 1024].rearrange(
                        "p (r q) -> p r q", q=N)
                    psrc = pre[:, :].rearrange("p (r q) -> p r q", q=N)
                    atb = at_sb[:, :].unsqueeze(1).broadcast_to([128, 4, N])
                    nc.vector.tensor_tensor(dst[:, :], psrc[:, :], atb,
                                            Alu.add)
                    if half == 1:
                        sil = lpool.tile([128, 2048], MDT, tag="sil",
                                         name=f"sil{pair}_{rp}")
                        nc.scalar.activation(sil[:], silin[:], Act.Silu,
                                             bias=b1c[:, 0:1])
                        sils[pair] = sil
                        # software-pipeline: emit W2 for the PREVIOUS pair so
                        # PE has the next pre-MMs queued while silu runs
                        if pair > 0:
                            emit_w2(pair - 1)
                        if pair == 31:
                            emit_w2(31)

                # ---- final: v = coeff @ pos - rowsum(coeff) * pos_p ----
                for t in range(2):
                    csb = fpool.tile([128, N], F32, tag="csb",
                                     name=f"csb{t}_{rp}")
                    nc.vector.tensor_scalar(csb[:], cps[:, N * t:N * (t + 1)],
                                            b2c[:, 0:1],
                                            None, Alu.add)
                    vcol = fpool.tile([128, 3], F32, tag="vcol",
                                      name=f"vcol{t}_{rp}")
                    scr = fpool.tile([128, N], F32, tag="scr",
                                     name=f"scr{t}_{rp}")
                    for a in range(3):
                        nc.vector.scalar_tensor_tensor(
                            scr[:], csb[:], 1.0, rep[a][:], Alu.mult, Alu.mult,
                            accum_out=vcol[:, a:a + 1])
                    rs = fpool.tile([128, 1], F32, tag="rs",
                                    name=f"rs{t}_{rp}")
                    nc.vector.tensor_scalar(scr[:], csb[:], 1.0, None,
                                            Alu.mult, Alu.add,
                                            accum_out=rs[:, 0:1])
                    rsp = fpool.tile([128, 3], F32, tag="rsp",
                                     name=f"rsp{t}_{rp}")
                    nc.gpsimd.tensor_scalar(rsp[:], pcol[t][:], rs[:, 0:1],
                                            None, Alu.mult)
                    vt = fpool.tile([128, 3], F32, tag="vt",
                                    name=f"vt{t}_{rp}")
                    nc.gpsimd.tensor_tensor(vt[:], vcol[:], rsp[:],
                                            Alu.subtract)
                    nc.sync.dma_start(v_d[128 * t:128 * (t + 1), :], vt[:])

    nc.compile()
    return nc


def _mdt_np():
    if MMDT == "bf16":
        import ml_dtypes
        return ml_dtypes.bfloat16
    return np.float32


def _prep_consts(W1, b1, W2, b2):
    mdt = _mdt_np()
    wa = np.ascontiguousarray(W1[:H]).astype(mdt)
    wb = np.ascontiguousarray(W1[H:2 * H]).astype(mdt)
    wd = W1[2 * H].astype(np.float32)
    wdrep = np.ascontiguousarray(np.tile(wd, 64)[None, :]).astype(mdt)
    ind = np.zeros((4, 2048), dtype=np.float32)
    cols = np.arange(2048)
    for j in range(4):
        ind[j, (cols // 256) % 4 == j] = 1.0
    ind = ind.astype(mdt)
    zw = np.zeros((H, 2 * H), dtype=np.float32)
    zw[:, H] = W2[:, 0]
    zw = zw.astype(mdt)
    b1c = np.ascontiguousarray(b1.reshape(H, 1), dtype=np.float32)
    b2c = np.full((128, 1), float(np.asarray(b2).reshape(-1)[0]),
                  dtype=np.float32)
    return dict(wa=wa, wb=wb, wdrep=wdrep, ind=ind, zw=zw, b1c=b1c, b2c=b2c)


def _make_in_maps(h, pos, consts):
    mdt = _mdt_np()
    in_maps = []
    for g in range(B):
        hg = h[g * N:(g + 1) * N]
        pg = pos[g * N:(g + 1) * N]
        rep3 = np.ascontiguousarray(
            np.broadcast_to(pg.T[:, None, :], (3, 128, N)), dtype=np.float32)
        m = {"hT": np.ascontiguousarray(hg.T).astype(mdt), "pos": pg,
             "rep3": rep3}
        m.update(consts)
        in_maps.append(m)
    return in_maps


def kernel(h, pos, batch, W1, b1, W2, b2, **unused):
    from concourse.bass_utils import run_bass_kernel_spmd

    h = np.ascontiguousarray(np.asarray(h, dtype=np.float32))
    pos = np.ascontiguousarray(np.asarray(pos, dtype=np.float32))
    W1 = np.asarray(W1, dtype=np.float32)
    b1 = np.asarray(b1, dtype=np.float32)
    W2 = np.asarray(W2, dtype=np.float32)
    b2 = np.asarray(b2, dtype=np.float32)

    if "nc" not in _cache:
        _cache["nc"] = _build()
    nc = _cache["nc"]

    consts = _prep_consts(W1, b1, W2, b2)
    in_maps = _make_in_maps(h, pos, consts)
    res = run_bass_kernel_spmd(nc, in_maps, core_ids=list(range(NCORES)))
    return np.concatenate([r["v"] for r in res.results], axis=0)



# revision 15
# speedup vs baseline: 1.3197x; 1.3197x over previous
"""Trainium2 Bass kernel for nn_EquivariantVelocityHead.

Full-input contract: kernel(**inputs) takes the unsharded inputs (as in
setup_inputs()) and returns the full [B*N, 3] output. Internally shards
data-parallel over the graph dimension B across 8 NeuronCores (all pairwise
interactions are intra-graph), with the tiny phi-MLP weights replicated.

Math (per graph, N=256 nodes, H=128):
  A = h @ W1[:H];  Bm = h @ W1[H:2H];  wd = W1[2H];  (phi layer 1 split)
  pre[p,q,:] = A[q] + Bm[p] + dist[p,q]*wd + b1
  coeff[p,q] = silu(pre) @ W2 + b2
  v[p] = sum_q coeff[p,q] * (pos[q] - pos[p])
       = coeff @ pos - rowsum(coeff) * pos[p]

Device layout: pre is materialized transposed [h=128 partitions, (p,q) free]
in PSUM by ONE K=128 bf16 matmul per 512 columns. The contraction packs all
three pre terms into the 128 rows via an SVD rotation of Wa: with
Wa = U S V^T, rows are [wd; B[u]; B[u+128]; (S V^T)[0:125]] on the
stationary side and [dist; ind_even; ind_odd; (h U)[:, 0:125]^T broadcast]
on the moving side. Dropping the 3 smallest singular directions of Wa costs
~2e-3 relative error on A (below the bf16 noise floor) and is what makes
the A-term fit: 1 + 2 + 125 = 128 = K. The PE is power-throttled when
overdriven, so halving its column count (vs recomputing A per block) is the
main lever. The per-u stationary [wd; B-rows; SV^T] lives in one wide tile
whose constant rows are replicated 128x by on-device log-doubling DMAs; the
moving tiles (8 persistent buffers of 4 node-slots) get their 125 h-rows
replicated the same way, with per-slot dist rows DMA-staged onto partition 0
and indicator rows selecting which half of each 512-block the B rows hit.
Silu+b1 is fused on ScalarE reading PSUM, writing bf16. The W2 contraction
uses a sliding-window stationary (zeros | W2-column | zeros) so node u's
pair accumulates into PSUM partition u, yielding coeff[p-part, q-free]
tiles; the final contraction runs on VectorE accumulating reduces.
"""
import sys

sys.path.insert(0, "/opt/trn_rl_repo")

import numpy as np

B, N, H = 8, 256, 128
NCORES = 8
KA = 125  # SVD-truncated rank of the Wa (A-term) contraction

_cache = {}


def _build(reps=1):
    import concourse.bacc as bacc
    import concourse.mybir as mybir
    import concourse.tile as tile

    F32 = mybir.dt.float32
    BF16 = mybir.dt.bfloat16
    Alu = mybir.AluOpType
    Act = mybir.ActivationFunctionType

    nc = bacc.Bacc()

    hT_d = nc.declare_dram_parameter("hT", [H, N], BF16, isOutput=False)
    htl_d = nc.declare_dram_parameter("htl", [KA, N], BF16, isOutput=False)
    pos_d = nc.declare_dram_parameter("pos", [N, 3], F32, isOutput=False)
    rep_d = nc.declare_dram_parameter("rep3", [3, 128, N], F32, isOutput=False)
    stat0_d = nc.declare_dram_parameter("stat0", [128, H], BF16,
                                        isOutput=False)
    wb_d = nc.declare_dram_parameter("wb", [H, H], BF16, isOutput=False)
    ind_d = nc.declare_dram_parameter("ind", [2, 2048], BF16, isOutput=False)
    zw_d = nc.declare_dram_parameter("zw", [H, 2 * H], BF16, isOutput=False)
    b1c_d = nc.declare_dram_parameter("b1c", [H, 1], F32, isOutput=False)
    b2c_d = nc.declare_dram_parameter("b2c", [128, 1], F32, isOutput=False)
    v_d = nc.declare_dram_parameter("v", [N, 3], F32, isOutput=True)

    with tile.TileContext(nc) as tc:
        with (
            tc.tile_pool(name="const", bufs=1) as cpool,
            tc.tile_pool(name="work", bufs=2) as wpool,
            tc.tile_pool(name="silu", bufs=4) as lpool,
            tc.tile_pool(name="fin", bufs=2) as fpool,
            tc.tile_pool(name="pre", bufs=3, space="PSUM") as pre_pool,
            tc.tile_pool(name="cps", bufs=1, space="PSUM") as cps_pool,
            tc.tile_pool(name="bps", bufs=1, space="PSUM") as bps_pool,
        ):
            # ---- constants / inputs ----
            hT = cpool.tile([H, N], BF16, tag="hT")
            nc.sync.dma_start(hT[:], hT_d[:])
            wb = cpool.tile([H, H], BF16, tag="wb")
            nc.sync.dma_start(wb[:], wb_d[:])
            zw = cpool.tile([H, 2 * H], BF16, tag="zw")
            nc.sync.dma_start(zw[:], zw_d[:])
            b1c = cpool.tile([H, 1], F32, tag="b1c")
            nc.sync.dma_start(b1c[:], b1c_d[:])
            b2c = cpool.tile([128, 1], F32, tag="b2c")
            nc.sync.dma_start(b2c[:], b2c_d[:])
            rep = []
            for a in range(3):
                r = cpool.tile([128, N], F32, tag=f"rep{a}")
                nc.sync.dma_start(r[:], rep_d[a])
                rep.append(r)
            pcol = []
            for t in range(2):
                p = cpool.tile([128, 3], F32, tag=f"pcol{t}")
                nc.sync.dma_start(p[:], pos_d[128 * t:128 * (t + 1), :])
                pcol.append(p)

            # ---- stationary tile [128 rows, 128 u-blocks x H]: row0 = wd,
            # rows 1-2 = per-u B pair (scattered later), rows 3.. = S V^T.
            # Constant rows replicated on-device by log-doubling DMAs. ----
            bwd = cpool.tile([128, 128 * H], BF16, tag="bwd")
            nc.sync.dma_start(bwd[:, 0:H], stat0_d[:])
            k = 1
            while k < 128:
                nc.sync.dma_start(bwd[:, k * H:2 * k * H], bwd[:, 0:k * H])
                k *= 2

            # ---- moving tiles: 8 persistent bufs of [128, 4*512]; rows
            # 3.. = (h U)^T replicated across the 8 q-chunks by doubling ----
            htl = cpool.tile([KA, N], BF16, tag="htl")
            nc.sync.dma_start(htl[:], htl_d[:])
            mov = []
            for bb in range(8):
                m = cpool.tile([128, 8 * N], BF16, tag=f"mov{bb}")
                nc.sync.dma_start(m[3:3 + KA, 0:N], htl[:])
                k = 1
                while k < 8:
                    nc.sync.dma_start(m[3:3 + KA, k * N:2 * k * N],
                                      m[3:3 + KA, 0:k * N])
                    k *= 2
                nc.sync.dma_start(m[1:3, :], ind_d[:, :])
                mov.append(m)

            for rp in range(reps):
                # ---- Bm = h @ Wb, scattered into bwd partitions 1-2 ----
                bsb = []
                for t in range(2):
                    bp = bps_pool.tile([128, H], F32, tag="bps")
                    nc.tensor.matmul(bp[:], hT[:, 128 * t:128 * (t + 1)],
                                     wb[:], start=True, stop=True,
                                     skip_group_check=True)
                    bs = wpool.tile([128, H], BF16, tag="bsb",
                                    name=f"bsb{t}_{rp}")
                    nc.vector.tensor_copy(bs[:], bp[:])
                    bsb.append(bs)
                for t in range(2):
                    dst = bwd[1 + t:2 + t, :]
                    nc.sync.dma_start(
                        dst.rearrange("o (a c) -> o a c", c=H), bsb[t][:])

                # ---- dist tiles [p-part, q-free], exact diff formulation ----
                dist = []
                for t in range(2):
                    dx = wpool.tile([128, N], F32, tag="dx", name=f"dx{t}_{rp}")
                    dy = wpool.tile([128, N], F32, tag="dy", name=f"dy{t}_{rp}")
                    dz = wpool.tile([128, N], F32, tag="dz", name=f"dz{t}_{rp}")
                    nc.vector.tensor_scalar(dx[:], rep[0][:], pcol[t][:, 0:1],
                                            None, Alu.subtract)
                    nc.vector.tensor_scalar(dy[:], rep[1][:], pcol[t][:, 1:2],
                                            None, Alu.subtract)
                    nc.vector.tensor_scalar(dz[:], rep[2][:], pcol[t][:, 2:3],
                                            None, Alu.subtract)
                    sx = wpool.tile([128, N], F32, tag="sx", name=f"sx{t}_{rp}")
                    sy = wpool.tile([128, N], F32, tag="sy", name=f"sy{t}_{rp}")
                    nc.vector.tensor_tensor(sx[:], dx[:], dx[:], Alu.mult)
                    nc.vector.tensor_tensor(sy[:], dy[:], dy[:], Alu.mult)
                    nc.vector.tensor_tensor(sx[:], sx[:], sy[:], Alu.add)
                    nc.vector.tensor_tensor(sy[:], dz[:], dz[:], Alu.mult)
                    nc.vector.tensor_tensor(sx[:], sx[:], sy[:], Alu.add)
                    dt_ = wpool.tile([128, N], BF16, tag="dist",
                                     name=f"dist{t}_{rp}")
                    nc.scalar.activation(dt_[:], sx[:], Act.Sqrt)
                    dist.append(dt_)

                # ---- stage dist rows onto partition 0 of the moving bufs:
                # stage c covers u-slots 4c..4c+3; slot layout
                # [dist0[u] (256) | dist1[u] (256)]. Emission must be
                # interleaved with the consuming matmuls: a buffer's next
                # write may only be emitted after the matmuls that read its
                # previous contents (program order defines the data flow). ----
                def emit_stage(c):
                    m = mov[c % 8]
                    row = m[0:1, :].rearrange("o (r two q) -> o r two q",
                                              two=2, q=N)
                    nc.sync.dma_start(row[:, :, 0, :],
                                      dist[0][4 * c:4 * c + 4, :])
                    nc.sync.dma_start(row[:, :, 1, :],
                                      dist[1][4 * c:4 * c + 4, :])

                for c in range(8):
                    emit_stage(c)

                # ---- coeff accumulator: cols 0:256 -> nodes 0..127,
                # cols 256:512 -> nodes 128..255 ----
                cps = cps_pool.tile([128, 2 * N], F32, tag="c",
                                    name=f"cps_{rp}")

                # ---- main loop: 64 blocks of 2 node-pairs ----
                def emit_w2(ob):
                    sil = sils[ob]
                    for hh in range(2):
                        u = 2 * ob + hh
                        nc.tensor.matmul(cps[:], zw[:, 128 - u:256 - u],
                                         sil[:, 512 * hh:512 * hh + 512],
                                         start=(u == 0), stop=(u == 127),
                                         skip_group_check=True)

                sils = {}
                for ob in range(64):
                    pre = pre_pool.tile([128, 1024], F32, tag="pre",
                                        name=f"pre{ob}_{rp}")
                    for hh in range(2):
                        u = 2 * ob + hh
                        m = mov[(u // 4) % 8]
                        rhs = m[0:128, (u % 4) * 512:(u % 4) * 512 + 512]
                        nc.tensor.matmul(pre[:, 512 * hh:512 * hh + 512],
                                         bwd[0:128, u * H:(u + 1) * H],
                                         rhs, start=True, stop=True,
                                         skip_group_check=True)
                    sil = lpool.tile([128, 1024], BF16, tag="sil",
                                     name=f"sil{ob}_{rp}")
                    nc.scalar.activation(sil[:], pre[:], Act.Silu,
                                         bias=b1c[:, 0:1])
                    sils[ob] = sil
                    # refill the stage buffer this block just finished with
                    if ob % 2 == 1 and ob // 2 + 8 < 32:
                        emit_stage(ob // 2 + 8)
                    # software-pipeline: emit W2 for the PREVIOUS block so PE
                    # has this block's pre-MMs queued while silu(ob-1) runs
                    if ob > 0:
                        emit_w2(ob - 1)
                    if ob == 63:
                        emit_w2(63)

                # ---- final: v = coeff @ pos - rowsum(coeff) * pos_p ----
                for t in range(2):
                    csb = fpool.tile([128, N], F32, tag="csb",
                                     name=f"csb{t}_{rp}")
                    nc.vector.tensor_scalar(csb[:], cps[:, N * t:N * (t + 1)],
                                            b2c[:, 0:1],
                                            None, Alu.add)
                    vcol = fpool.tile([128, 3], F32, tag="vcol",
                                      name=f"vcol{t}_{rp}")
                    scr = fpool.tile([128, N], F32, tag="scr",
                                     name=f"scr{t}_{rp}")
                    for a in range(3):
                        nc.vector.scalar_tensor_tensor(
                            scr[:], csb[:], 1.0, rep[a][:], Alu.mult, Alu.mult,
                            accum_out=vcol[:, a:a + 1])
                    rs = fpool.tile([128, 1], F32, tag="rs",
                                    name=f"rs{t}_{rp}")
                    nc.vector.tensor_scalar(scr[:], csb[:], 1.0, None,
                                            Alu.mult, Alu.add,
                                            accum_out=rs[:, 0:1])
                    rsp = fpool.tile([128, 3], F32, tag="rsp",
                                     name=f"rsp{t}_{rp}")
                    nc.vector.tensor_scalar(rsp[:], pcol[t][:], rs[:, 0:1],
                                            None, Alu.mult)
                    vt = fpool.tile([128, 3], F32, tag="vt",
                                    name=f"vt{t}_{rp}")
                    nc.vector.tensor_tensor(vt[:], vcol[:], rsp[:],
                                            Alu.subtract)
                    nc.sync.dma_start(v_d[128 * t:128 * (t + 1), :], vt[:])

    nc.compile()
    return nc


def _mdt_np():
    import ml_dtypes
    return ml_dtypes.bfloat16


def _prep_consts(W1, b1, W2, b2):
    mdt = _mdt_np()
    Wa = np.ascontiguousarray(W1[:H], dtype=np.float64)
    U, S, Vt = np.linalg.svd(Wa)
    uproj = U[:, :KA].astype(np.float32)                   # [H, KA]
    svt = (S[:KA, None] * Vt[:KA]).astype(np.float32)      # [KA, H]
    wd = W1[2 * H].astype(np.float32)
    stat0 = np.zeros((128, H), dtype=np.float32)
    stat0[0] = wd
    stat0[3:3 + KA] = svt
    stat0 = stat0.astype(mdt)
    wb = np.ascontiguousarray(W1[H:2 * H]).astype(mdt)
    ind = np.zeros((2, 2048), dtype=np.float32)
    cols = np.arange(2048)
    ind[0, (cols % 512) < 256] = 1.0
    ind[1, (cols % 512) >= 256] = 1.0
    ind = ind.astype(mdt)
    zw = np.zeros((H, 2 * H), dtype=np.float32)
    zw[:, H] = W2[:, 0]
    zw = zw.astype(mdt)
    b1c = np.ascontiguousarray(b1.reshape(H, 1), dtype=np.float32)
    b2c = np.full((128, 1), float(np.asarray(b2).reshape(-1)[0]),
                  dtype=np.float32)
    consts = dict(stat0=stat0, wb=wb, ind=ind, zw=zw, b1c=b1c, b2c=b2c)
    return consts, uproj


def _make_in_maps(h, pos, consts, uproj):
    mdt = _mdt_np()
    in_maps = []
    for g in range(B):
        hg = h[g * N:(g + 1) * N]
        pg = pos[g * N:(g + 1) * N]
        rep3 = np.ascontiguousarray(
            np.broadcast_to(pg.T[:, None, :], (3, 128, N)), dtype=np.float32)
        htl = np.ascontiguousarray((hg @ uproj).T)          # [KA, N]
        m = {"hT": np.ascontiguousarray(hg.T).astype(mdt),
             "htl": htl.astype(mdt), "pos": pg, "rep3": rep3}
        m.update(consts)
        in_maps.append(m)
    return in_maps


def kernel(h, pos, batch, W1, b1, W2, b2, **unused):
    from concourse.bass_utils import run_bass_kernel_spmd

    h = np.ascontiguousarray(np.asarray(h, dtype=np.float32))
    pos = np.ascontiguousarray(np.asarray(pos, dtype=np.float32))
    W1 = np.asarray(W1, dtype=np.float32)
    b1 = np.asarray(b1, dtype=np.float32)
    W2 = np.asarray(W2, dtype=np.float32)
    b2 = np.asarray(b2, dtype=np.float32)

    if "nc" not in _cache:
        _cache["nc"] = _build()
    nc = _cache["nc"]

    consts, uproj = _prep_consts(W1, b1, W2, b2)
    in_maps = _make_in_maps(h, pos, consts, uproj)
    res = run_bass_kernel_spmd(nc, in_maps, core_ids=list(range(NCORES)))
    return np.concatenate([r["v"] for r in res.results], axis=0)
